# revision 27
# baseline (speedup 1.0000x reference)
"""Trainium2 Bass kernel for CausalWanSelfAttention (KV-cache-bias attention).

Math: the reference's disjoint-segment attention + LSE merge is exactly
global softmax with a per-key bias b_l (log 0.1 on keys in
[frame_seqlen, current_block_start)).  exp needs no max-subtraction
(scores ~ N(0,1), max ~ 6), so out = (E @ V) / (1^T E) with
E = exp(scale*S + b_l) — the bias folds into the ACT exp as a
per-partition bias (partition = key index within the 128-chunk).

Sharding: 24 units = (head h in 0..11, q-half in {0,1}), 3 units per core.
Each unit: 1024 queries x 1 head x all 8192 keys, 64 key chunks of 128.

The kernel is ACT-bound: 192 exp instrs x [128x1024] ~ 199us busy per
core (1 elem/lane/cycle at 1.2 GHz, no fast mode exists for ACT), so
everything else is shaped to hide under the exp stream:
  A:    S^T[l 128, q 1024] = kt-chunk^T @ qt     (MMW_A matmuls, bf16)
  exp:  E = exp(S^T * scale + bias_l) bf16       (1 ACT instr / chunk)
  B:    O^T[d 128, q 1024] += v-chunk^T @ E      (MMW_B matmuls)
  norm: n[1, q 1024] += ones^T @ (hex-tree sum)  (2 MM per 16 chunks)
- Norm hex-tree: DVE adds E pairs->quads->octs->hex, one level per
  B-slot (staggered so no slot carries >2 adds); PE then streams only
  NLC/16 norm matmul pairs per unit.  The stream's final 16 chunks use
  quad/pair/e-direct fast paths so the post-exp drain chain is short.
- MMW_A = MMW_B = 4 (256-wide matmuls): HW-measured optimum.  Paired
  TIME_LOOP-slope timing gives body 233us at width 4 vs 250us at width
  2 and 310us at width 8 (cost model is width-blind at 213us; ldweights
  count does NOT hurt - the 2x512 version is slower despite fewer
  loads).  PSUM start/stop flags are per 2KB zero-region: only the
  first/last 256-slice of each bank carries start/stop.
- PSUM budget (8 banks): S double-buffered (4) + O^T accum (2) +
  norm accum (2).  fp8 phase-B was evaluated and rejected: E or V in
  e4m3 alone costs ~2.8e-2 rel err vs the 2e-2 gate.
- Unit-0 DMAs are ordered critical-path-first (kt head chunk, qt
  halves, bias) so A(0) issues ~2.4us in; units 1,2 prefetch with big
  DMAs ~56 chunks ahead.  A dep-free 1x1 dummy matmul warms the PE
  p-state ramp during the first DMA wait.
Final divide by n and the [d,q]->[q,d] transpose happen host-side on the
fp32 partials (exact).
"""

import math
import sys

for _p in ("/opt/trn_rl_repo",):
    if _p not in sys.path:
        sys.path.insert(0, _p)

import numpy as np
import ml_dtypes

import concourse.bass as bass
import concourse.mybir as mybir
import concourse.tile as tile
from concourse import bacc
from concourse.bass_utils import run_bass_kernel_spmd

BF16 = mybir.dt.bfloat16
F32 = mybir.dt.float32
NP_BF16 = ml_dtypes.bfloat16

B, LQ, LK, H, D = 1, 2048, 8192, 12, 128
N_CORES = 8
UNITS_PER_CORE = 3          # 24 units = 12 heads x 2 q-halves
QSPAN = 1024                # queries per unit
NLC = LK // 128             # 64 key chunks of 128
SCALE = 1.0 / math.sqrt(D)

_CACHED = None
ABLATE = "base"   # timing experiments only; "base" is the real kernel
TIME_LOOP = 1     # timing experiments only: hardware-loop the body N times
MMW_A = 4         # matmuls per 1024-wide A phase (HW-tuned; see docstring)
MMW_B = 4         # matmuls per 1024-wide B phase


def _build_program():
    nc = bacc.Bacc("TRN2", target_bir_lowering=False, debug=False,
                   enable_asserts=False)

    qt_d = nc.dram_tensor("qt", [UNITS_PER_CORE, 128, QSPAN], BF16,
                          kind="ExternalInput")
    kt_d = nc.dram_tensor("kt", [UNITS_PER_CORE, 128, LK], BF16,
                          kind="ExternalInput")
    vl_d = nc.dram_tensor("vl", [UNITS_PER_CORE, LK, 128], BF16,
                          kind="ExternalInput")
    bias_d = nc.dram_tensor("bias", [128, NLC], F32, kind="ExternalInput")
    ot_d = nc.dram_tensor("ot", [UNITS_PER_CORE, 128, QSPAN], F32,
                          kind="ExternalOutput")
    nm_d = nc.dram_tensor("nm", [UNITS_PER_CORE, 1, QSPAN], F32,
                          kind="ExternalOutput")

    qt_ap = qt_d.ap()
    kt_ap = kt_d.ap()
    # [u, (c p), d] -> [u, p, c, d]: partition = key index within chunk
    vl_ap = vl_d.ap().rearrange("u (c p) d -> u p c d", p=128)
    bias_ap = bias_d.ap()
    ot_ap = ot_d.ap()
    nm_ap = nm_d.ap()

    with tile.TileContext(nc) as tc:
        with (
            tc.tile_pool(name="kt_pool", bufs=2) as kt_pool,
            tc.tile_pool(name="vl_pool", bufs=2) as vl_pool,
            tc.tile_pool(name="qt_pool", bufs=2) as qt_pool,
            tc.tile_pool(name="cn_pool", bufs=1) as cn_pool,
            tc.tile_pool(name="e_pool", bufs=6) as e_pool,
            tc.tile_pool(name="ob_pool", bufs=2) as ob_pool,
            tc.tile_pool(name="s_pool", bufs=2, space="PSUM") as s_pool,
            tc.tile_pool(name="o_pool", bufs=1, space="PSUM") as o_pool,
            tc.tile_pool(name="n_pool", bufs=1, space="PSUM") as n_pool,
        ):
            bias_t = cn_pool.tile([128, NLC], F32, name="bias_t")
            ones_t = cn_pool.tile([128, 1], BF16, name="ones_t")

            import contextlib
            loop_cm = (tc.For_i(0, TIME_LOOP, 1) if TIME_LOOP > 1
                       else contextlib.nullcontext())

            loaded = {}

            def load_unit0():
                # Critical-path-first DMA order so A(0)/exp(0) start ASAP:
                # kt head chunk, qt, bias, then the bulk in pieces.
                u = 0
                qt = qt_pool.tile([128, QSPAN], BF16, name=f"qt_u{u}", tag="qt")
                kt = kt_pool.tile([128, LK], BF16, name=f"kt_u{u}", tag="kt")
                vl = vl_pool.tile([128, NLC, 128], BF16,
                                  name=f"vl_u{u}", tag="vl")
                nc.sync.dma_start(out=kt[:, 0:256], in_=kt_ap[u][:, 0:256])
                for half in range(2):
                    sl = bass.ts(half, QSPAN // 2)
                    nc.sync.dma_start(out=qt[:, sl], in_=qt_ap[u][:, sl])
                nc.sync.dma_start(out=bias_t[:], in_=bias_ap)
                nc.vector.memset(ones_t[:], 1.0)
                nc.sync.dma_start(out=kt[:, 256:1024], in_=kt_ap[u][:, 256:1024])
                nc.sync.dma_start(out=vl[:, 0:8, :], in_=vl_ap[u][:, 0:8, :])
                for eighth in range(1, 8):
                    slk = bass.ts(eighth, LK // 8)
                    nc.sync.dma_start(out=kt[:, slk], in_=kt_ap[u][:, slk])
                    slv = bass.ts(eighth, NLC // 8)
                    nc.sync.dma_start(out=vl[:, slv, :], in_=vl_ap[u][:, slv, :])
                loaded[u] = (kt, vl, qt)

            def load_unit(u):
                # Prefetched ~56 chunks ahead of use: few big DMAs suffice.
                qt = qt_pool.tile([128, QSPAN], BF16, name=f"qt_u{u}", tag="qt")
                kt = kt_pool.tile([128, LK], BF16, name=f"kt_u{u}", tag="kt")
                vl = vl_pool.tile([128, NLC, 128], BF16,
                                  name=f"vl_u{u}", tag="vl")
                nc.sync.dma_start(out=qt[:], in_=qt_ap[u])
                for half in range(2):
                    slk = bass.ts(half, LK // 2)
                    nc.sync.dma_start(out=kt[:, slk], in_=kt_ap[u][:, slk])
                    slv = bass.ts(half, NLC // 2)
                    nc.sync.dma_start(out=vl[:, slv, :], in_=vl_ap[u][:, slv, :])
                loaded[u] = (kt, vl, qt)

            NG = UNITS_PER_CORE * NLC
            HEX = 16                # chunks per norm hex-group

            with loop_cm:
                load_unit0()
                # One global software-pipelined chunk stream across all
                # units: emit A(g) before B(g-1) so PE's in-order queue
                # always has independent work while ACT runs exp(g-1), and
                # the next unit's A-phase fills the previous unit's drain.
                # Norm: DVE reduces E tiles 16->1 via a pair/quad/oct/hex
                # add tree whose levels are staggered one B-slot apart
                # (pair at +0, quad +1, oct +2, hex +3 relative to the
                # group's last B), so no slot carries more than ~2 DVE adds
                # and the two 512-wide norm matmuls land in different slots
                # (+4, +5).  The very last hex group of the stream skips
                # oct/hex and feeds the norm from its two quad tiles, which
                # shortens the end-of-kernel drain chain.  PSUM accumulators
                # (ot/nm, single-buffered) are allocated lazily at first
                # write so the pool rotation lands after the previous
                # unit's evacuation instr is emitted.
                LASTH = NG - HEX    # first chunk of the final hex group
                cur, ot_t, nm_t = {}, {}, {}
                etiles, ptiles, qtiles, otiles, htiles = {}, {}, {}, {}, {}

                def norm_mm(un, nl, src, start, stop, halves=(0, 1)):
                    if start and un not in nm_t:
                        nm_t[un] = n_pool.tile([128, QSPAN], F32,
                                               name=f"nm_u{un}", tag="nm")
                    for half in halves:
                        sl = bass.ts(half, 512)
                        nc.tensor.matmul(
                            nm_t[un][0:1, sl], lhsT=ones_t[:],
                            rhs=src[:, sl], start=start, stop=stop)

                def nm_evac(un):
                    nm = nm_t.pop(un)
                    nm_sb = ob_pool.tile([1, QSPAN], F32,
                                         name=f"nmsb_u{un}", tag="nmsb")
                    nc.vector.tensor_scalar_add(nm_sb[:], nm[0:1, :], 0.0)
                    nc.sync.dma_start(out=nm_ap[un], in_=nm_sb[:])

                warmed = False
                for g in range(NG + 6):
                    if g < NG:
                        ug, cg = g // NLC, g % NLC
                        if cg == 0:
                            cur[ug] = loaded.pop(ug)
                        kt, vl, qt = cur[ug]
                        s = s_pool.tile([128, QSPAN], F32)
                        if not warmed:
                            # dep-free dummy matmul: starts the PE p-state
                            # ramp while the first DMAs are in flight, so
                            # A(0) runs at a warmer clock
                            nc.tensor.matmul(
                                s[0:1, 0:1], lhsT=ones_t[:, 0:1],
                                rhs=ones_t[:, 0:1], start=True, stop=True,
                                skip_group_check=True)
                            warmed = True
                        for half in range(MMW_A):
                            sl = bass.ts(half, QSPAN // MMW_A)
                            p_ = half % max(1, MMW_A // 2)
                            nc.tensor.matmul(
                                s[:, sl], lhsT=kt[:, bass.ts(cg, 128)],
                                rhs=qt[:, sl],
                                start=(p_ == 0),
                                stop=(p_ == max(1, MMW_A // 2) - 1))
                        e = e_pool.tile([128, QSPAN], BF16)
                        nc.scalar.activation(
                            e[:], s[:], mybir.ActivationFunctionType.Exp,
                            bias=bias_t[:, cg:cg + 1], scale=SCALE)
                        etiles[g] = e
                        if cg == 8 and ug + 1 < UNITS_PER_CORE:
                            load_unit(ug + 1)  # prefetch next unit's inputs
                    d = g - 1               # chunk whose B phase is due
                    if 0 <= d < NG:
                        ud, dl = d // NLC, d % NLC
                        if dl == 0:
                            ot_t[ud] = o_pool.tile([128, QSPAN], F32,
                                                   name=f"ot_u{ud}", tag="ot")
                        e = etiles[d]
                        for half in range(MMW_B):
                            sl = bass.ts(half, QSPAN // MMW_B)
                            p_ = half % max(1, MMW_B // 2)
                            nc.tensor.matmul(
                                ot_t[ud][:, sl], lhsT=cur[ud][1][:, dl, :],
                                rhs=e[:, sl],
                                start=(dl == 0 and p_ == 0),
                                stop=(dl == NLC - 1
                                      and p_ == max(1, MMW_B // 2) - 1))
                        if d >= NG - 2:
                            # final two chunks: norm directly off the e
                            # tile right after its B — the drain chain
                            # after the last exp is then PE-only
                            un, nl = d // NLC, d % NLC
                            norm_mm(un, nl, e, start=False,
                                    stop=(d == NG - 1))
                            etiles.pop(d)
                            if d == NG - 1:
                                nm_evac(un)
                        elif d >= NG - 4 and dl % 2 == 1:
                            # chunks NG-4, NG-3: norm off the pair tile
                            un, nl = d // NLC, d % NLC
                            pp = e_pool.tile([128, QSPAN], BF16,
                                             tag="pp", name=f"pp_{d}")
                            nc.vector.tensor_add(
                                pp[:], etiles.pop(d - 1)[:],
                                etiles.pop(d)[:])
                            norm_mm(un, nl, pp, start=False, stop=False)
                        elif dl % 2 == 1:
                            pp = e_pool.tile([128, QSPAN], BF16,
                                             tag="pp", name=f"pp_{d}")
                            nc.vector.tensor_add(
                                pp[:], etiles.pop(d - 1)[:],
                                etiles.pop(d)[:])
                            ptiles[d // 2] = pp
                        if dl == NLC - 1:
                            ot = ot_t.pop(ud)
                            for half in range(2):
                                sl = bass.ts(half, 512)
                                ot_sb = ob_pool.tile(
                                    [128, 512], F32, bufs=4,
                                    name=f"otsb_u{ud}_{half}", tag="otsb")
                                nc.vector.tensor_scalar_add(
                                    ot_sb[:], ot[:, sl], 0.0)
                                nc.sync.dma_start(out=ot_ap[ud][:, sl],
                                                  in_=ot_sb[:])
                    q4 = g - 2              # chunk whose quad add is due
                    if 0 <= q4 < NG and q4 % 4 == 3 and q4 < NG - 4:
                        qq = e_pool.tile([128, QSPAN], BF16,
                                         tag="qq", name=f"qq_{q4}")
                        nc.vector.tensor_add(
                            qq[:], ptiles.pop(q4 // 2 - 1)[:],
                            ptiles.pop(q4 // 2)[:])
                        if q4 < LASTH:
                            qtiles[q4 // 4] = qq
                        else:
                            # final hex group of the stream: feed norm
                            # straight from each quad (skip oct/hex) so the
                            # drain chain after the last exp stays short
                            un, nl = q4 // NLC, q4 % NLC
                            norm_mm(un, nl, qq, start=False, stop=False)
                    o8 = g - 3              # chunk whose oct add is due
                    if 0 <= o8 < NG and o8 % 8 == 7 and o8 < LASTH:
                        oo = e_pool.tile([128, QSPAN], BF16,
                                         tag="oo", name=f"oo_{o8}", bufs=2)
                        nc.vector.tensor_add(
                            oo[:], qtiles.pop(o8 // 4 - 1)[:],
                            qtiles.pop(o8 // 4)[:])
                        otiles[o8 // 8] = oo
                    h16 = g - 4             # chunk whose hex add is due
                    if 0 <= h16 < NG and h16 % HEX == HEX - 1 and h16 < LASTH:
                        hh = e_pool.tile([128, QSPAN], BF16,
                                         tag="hh", name=f"hh_{h16}", bufs=2)
                        nc.vector.tensor_add(
                            hh[:], otiles.pop(h16 // 8 - 1)[:],
                            otiles.pop(h16 // 8)[:])
                        htiles[h16 // HEX] = hh
                    # regular norm matmul halves, spread over 2 slots
                    for half in range(2):
                        n = g - 5 - half
                        if (0 <= n < NG and n % HEX == HEX - 1
                                and n < LASTH):
                            un, nl = n // NLC, n % NLC
                            norm_mm(un, nl, htiles[n // HEX],
                                    start=(nl == HEX - 1),
                                    stop=(nl == NLC - 1), halves=(half,))
                            if half == 1:
                                htiles.pop(n // HEX)
                                if nl == NLC - 1:
                                    nm_evac(un)

    nc.compile()
    return nc


def _get_program():
    global _CACHED
    if _CACHED is None:
        _CACHED = _build_program()
    return _CACHED


def _host_prep(q, k, v, frame_seqlen, current_block_start):
    fs = max(0, min(int(frame_seqlen), LK))
    bs = max(0, min(int(current_block_start), LK))
    logw = np.zeros(LK, np.float32)
    logw[fs:bs] = math.log(0.1)
    bias = np.ascontiguousarray(logw.reshape(NLC, 128).T)  # [128, NLC]

    q = np.asarray(q, dtype=np.float32)
    k = np.asarray(k, dtype=np.float32)
    v = np.asarray(v, dtype=np.float32)

    qT = np.ascontiguousarray(q[0].transpose(1, 2, 0)).astype(NP_BF16)  # [H,128,LQ]
    kT = np.ascontiguousarray(k[0].transpose(1, 2, 0)).astype(NP_BF16)  # [H,128,LK]
    vL = np.ascontiguousarray(v[0].transpose(1, 0, 2)).astype(NP_BF16)  # [H,LK,128]

    in_maps = []
    for i in range(N_CORES):
        units = [3 * i + uu for uu in range(UNITS_PER_CORE)]
        heads = [g // 2 for g in units]
        qhs = [g % 2 for g in units]
        in_maps.append({
            "qt": np.ascontiguousarray(
                np.stack([qT[h, :, qh * QSPAN:(qh + 1) * QSPAN]
                          for h, qh in zip(heads, qhs)])),
            "kt": np.ascontiguousarray(np.stack([kT[h] for h in heads])),
            "vl": np.ascontiguousarray(np.stack([vL[h] for h in heads])),
            "bias": bias,
        })
    return in_maps


def _assemble(results):
    out = np.empty((B, LQ, H, D), np.float32)
    for i in range(N_CORES):
        ot = results[i]["ot"]   # [3, 128, 1024] unnormalized O^T
        nm = results[i]["nm"][:, 0]   # [3, 1024]
        for uu in range(UNITS_PER_CORE):
            g = 3 * i + uu
            h, qh = g // 2, g % 2
            out[0, qh * QSPAN:(qh + 1) * QSPAN, h, :] = (
                ot[uu] / nm[uu][None, :]).T
    return out


def kernel(q, k, v, frame_seqlen, current_block_start):
    nc = _get_program()
    in_maps = _host_prep(q, k, v, frame_seqlen, current_block_start)
    res = run_bass_kernel_spmd(nc, in_maps, core_ids=list(range(N_CORES)))
    return _assemble(res.results)

